# revision 6
# baseline (speedup 1.0000x reference)
"""Trainium2 Bass kernel for nn_Gate_Net (Toeplitz + hard-sigmoid prob + cumprod gate).

Reference computation (per document row of 1024 scores):
  s = doc[1:-1]                      # n = 1022
  score_hat[r, j] = s[j-1-r] if j-1-r >= 0 else 0      # [1021, 1022]
  p[r, j] = clamp(10*(score_hat - s[j]) + 1, 0, 1)      # hard branch, res=0.1
  fwd = cumprod(p, axis=0); bwd = same with s reversed
  out = stack([fwd, bwd]) per doc -> full [32, 2, 1021, 1022] f32

Device algorithm (per doc-dir, column-major, transpose-free):
  Column j's factor sequence over rows m is
    q(j, m) = clamp(g[j-1-m] + c_j, 0, 1),  g[x] = 10*s[x] (0 outside [0,n)),
    c_j = 1 - 10*s[j].
  One band DMA per doc-dir materializes Ball[p, t] = arrR[p + t]
  (arrR = [0, reversed(10*s), zeros]; partition p holds column
  jb*128 + 127 - p -- the host un-reverses for free) so that for every
  128-column block jb the factor matrix [col-part p, row-free m] is the
  uniform slice Ball[:, 896-jb*128 : 896-jb*128+1021] (zeros beyond the data
  edge give the boundary/tail factors automatically).  Then per jb:
    tensor_scalar(add c, min 1) -> Relu -> tensor_tensor_scan (cumprod along
  the free/row axis, fp32 state, bf16 downcast out).  The scan result is
  already [col, row] = the transpose of the output, so a single 3D-AP DMA per
  doc-dir stores it column-major to DRAM; the host transposes (memory-bound
  problem: bf16 halves HBM store traffic, rel-err ~2e-3 << 2e-2 gate).

Sharding: pure data parallel, 8 doc-dirs per core (4 docs x fwd/bwd).
"""
import numpy as np

import concourse.bass as bass
import concourse.bacc as bacc
import concourse.tile as tile
from concourse import mybir
from concourse import bass_utils

P = 128
N = 1022          # columns j per doc-dir
ROWS = N - 1      # 1021 output rows (m axis)
NB = 8            # column blocks
ARRW = 2044       # reversed band source width
BALLW = 1917      # band tile width: max slice offset 896 + 1021

_NC_CACHE: dict = {}


def build_nc(n_dd: int = 8):
    """Single-core Bass program processing n_dd doc-dirs, column-major out."""
    nc = bacc.Bacc("TRN2", target_bir_lowering=False, debug=False, num_devices=8)
    arr = nc.dram_tensor("arr", [n_dd, ARRW], mybir.dt.float32, kind="ExternalInput")
    cc = nc.dram_tensor("cc", [P, n_dd * NB], mybir.dt.float32, kind="ExternalInput")
    out = nc.dram_tensor(
        "out", [n_dd, NB, P, ROWS], mybir.dt.bfloat16, kind="ExternalOutput"
    )

    add_op = mybir.AluOpType.add
    min_op = mybir.AluOpType.min
    mult_op = mybir.AluOpType.mult
    relu = mybir.ActivationFunctionType.Relu

    with tile.TileContext(nc) as tc:
        with (
            tc.tile_pool(name="consts", bufs=1) as consts,
            tc.tile_pool(name="band", bufs=2) as band_pool,
            tc.tile_pool(name="qpool", bufs=3) as qpool,
            tc.tile_pool(name="rpool", bufs=2) as rpool,
        ):
            zeros = consts.tile([P, ROWS], mybir.dt.float32)
            nc.vector.memset(zeros[:], 0.0)
            csb = consts.tile([P, n_dd * NB], mybir.dt.float32)
            nc.sync.dma_start(out=csb[:], in_=cc[:, :])

            for dd in range(n_dd):
                Ball = band_pool.tile([P, BALLW], mybir.dt.float32, tag="Ball")
                band_src = bass.AP(
                    tensor=arr, offset=dd * ARRW, ap=[[1, P], [1, BALLW]]
                )
                nc.sync.dma_start(out=Ball[:], in_=band_src)

                R = rpool.tile([P, NB * ROWS], mybir.dt.bfloat16, tag="R", name="R")
                for jb in range(NB):
                    off = 896 - jb * 128
                    Q = qpool.tile([P, ROWS], mybir.dt.float32, tag="Q", name="Q")
                    nc.vector.tensor_scalar(
                        out=Q[:],
                        in0=Ball[:, off:off + ROWS],
                        scalar1=csb[:, dd * NB + jb:dd * NB + jb + 1],
                        scalar2=1.0,
                        op0=add_op,
                        op1=min_op,
                    )
                    nc.scalar.activation(
                        out=Q[:], in_=Q[:], func=relu, bias=0.0, scale=1.0
                    )
                    nc.vector.tensor_tensor_scan(
                        out=R[:, jb * ROWS:(jb + 1) * ROWS],
                        data0=Q[:],
                        data1=zeros[:],
                        initial=1.0,
                        op0=mult_op,
                        op1=add_op,
                    )
                dst = bass.AP(
                    tensor=out,
                    offset=dd * NB * P * ROWS,
                    ap=[[ROWS, P], [P * ROWS, NB], [1, ROWS]],
                )
                nc.sync.dma_start(out=dst, in_=R[:])
    nc.compile()
    return nc


def get_nc(n_dd: int = 8):
    if n_dd not in _NC_CACHE:
        _NC_CACHE[n_dd] = build_nc(n_dd)
    return _NC_CACHE[n_dd]


def make_core_inputs(docs_core: np.ndarray) -> dict:
    """docs_core: [n_docs, 1024] f32 -> in_map with arr/cc for n_docs*2 doc-dirs."""
    n_docs = docs_core.shape[0]
    n_dd = n_docs * 2
    arr = np.zeros((n_dd, ARRW), np.float32)
    cc = np.ones((P, n_dd * NB), np.float32)
    for dl in range(n_docs):
        s = docs_core[dl, 1:-1].astype(np.float32)  # 1022
        for t in range(2):
            v = s if t == 0 else s[::-1]
            dd = dl * 2 + t
            v10 = (np.float32(10.0) * v).astype(np.float32)
            arr[dd, 1:1 + N] = v10[::-1]
            cvals = np.ones(NB * P, np.float32)
            cvals[:N] = np.float32(1.0) - v10
            # partition p holds column jb*128 + 127 - p
            cc[:, dd * NB:(dd + 1) * NB] = cvals.reshape(NB, P)[:, ::-1].T
    return {"arr": arr, "cc": cc}


def kernel(score: np.ndarray, score_idx: np.ndarray) -> np.ndarray:
    score = np.asarray(score, dtype=np.float32)
    score_idx = np.asarray(score_idx)
    docs = score[score_idx]  # [B, L] gather
    Bn, L = docs.shape       # 32, 1024
    n_cores = 8
    docs_per_core = Bn // n_cores  # 4

    in_maps = [
        make_core_inputs(docs[c * docs_per_core:(c + 1) * docs_per_core])
        for c in range(n_cores)
    ]
    nc = get_nc(docs_per_core * 2)
    res = bass_utils.run_bass_kernel_spmd(nc, in_maps, core_ids=list(range(n_cores)))
    full = np.empty((Bn, 2, ROWS, N), np.float32)
    for c in range(n_cores):
        o = np.asarray(res.results[c]["out"])  # [n_dd, NB, P, ROWS] bf16
        # partition p holds column jb*128 + 127 - p: un-reverse blocks
        o = o[:, :, ::-1, :].reshape(docs_per_core * 2, NB * P, ROWS)
        o = o.astype(np.float32)
        for dl in range(docs_per_core):
            for t in range(2):
                dd = dl * 2 + t
                full[c * docs_per_core + dl, t] = o[dd].T[:, :N]
    return full
